# revision 10
# baseline (speedup 1.0000x reference)
"""Gaussian RBF network kernel for 8 Trainium2 NeuronCores.

Computes out[n] = sum_c w[c] * exp(-0.5 * (x_n - c_c)^T P (x_n - c_c)),
P = L @ L.T from packed lower-triangular elements, N=8192, C=512, F=128.

Strategy: data-parallel over N (1024 rows per core), everything in
transposed orientation so the final weighted reduction over centers is a
matmul over the partition axis. q_x / q_c fold into the main cross matmul
via a K=2 PSUM-accumulate matmul:
  A[c, n] = sum_k CT[k,c] W[k,n] + 1 * (-0.5 q_x[n]) + (-0.5 q_c[c]) * 1
          = -0.5 (x_n - c_c)^T P (x_n - c_c)
  PhiT = exp(A);  out[n] = sum_c wcol[c] PhiT[c, n]   (w-as-lhsT matmuls)
Matmul operands live as float32r (full-rate fp32 on the PE).
"""

import contextlib
import ctypes
import sys
import types

import numpy as np

N, C, F = 8192, 512, 128
NCORES = 8
NC = N // NCORES  # rows per core
NH = NC // 512  # 512-column n-chunks per core
CT_TILES = C // 128

_cache = {}


def _install_ntff_hook():
    """bass_utils wants antenv.axon_hooks for trace=True under axon; the
    image lacks it. Provide the same ctypes hook trn_boot would install.
    Degrades silently if anything is off (tracing just gets skipped)."""
    if "antenv.axon_hooks" in sys.modules:
        return
    try:
        import antenv

        so_path = "/opt/axon/libaxon_pjrt.so"
        lib = ctypes.CDLL(so_path)
        if not hasattr(lib, "axon_start_nrt_profile"):
            return
        lib.axon_start_nrt_profile.argtypes = [
            ctypes.POINTER(ctypes.c_int64),
            ctypes.c_size_t,
        ]
        lib.axon_start_nrt_profile.restype = ctypes.c_int64
        lib.axon_stop_nrt_profile.argtypes = [ctypes.c_char_p]
        lib.axon_stop_nrt_profile.restype = ctypes.c_int64

        @contextlib.contextmanager
        def _hook(output_dir, device_ids):
            import jax

            jax.devices()
            if device_ids:
                ids = (ctypes.c_int64 * len(device_ids))(*device_ids)
                rc = lib.axon_start_nrt_profile(ids, len(device_ids))
            else:
                rc = lib.axon_start_nrt_profile(None, 0)
            if rc != 0:
                raise RuntimeError(f"axon_start_nrt_profile rc={rc}")
            try:
                yield
            finally:
                n = lib.axon_stop_nrt_profile(str(output_dir).encode())
                if n < 0:
                    raise RuntimeError(f"axon_stop_nrt_profile rc={n}")

        mod = types.ModuleType("antenv.axon_hooks")
        mod.get_axon_ntff_profile_hook = lambda: _hook
        mod.set_axon_ntff_profile_hook = lambda h: None
        sys.modules["antenv.axon_hooks"] = mod
        antenv.axon_hooks = mod
    except Exception:
        pass


def _build():
    import concourse.bass as bass
    import concourse.mybir as mybir
    import concourse.tile as tile
    from concourse import bacc

    f32 = mybir.dt.float32
    bf16 = mybir.dt.bfloat16
    Exp = mybir.ActivationFunctionType.Exp

    nc = bacc.Bacc(
        "TRN2", target_bir_lowering=False, debug=False, num_devices=NCORES
    )
    xt_d = nc.dram_tensor("xt", [F, NC], bf16, kind="ExternalInput")
    u_d = nc.dram_tensor("u", [F, F], bf16, kind="ExternalInput")
    ct_d = nc.dram_tensor("ct", [F, C], bf16, kind="ExternalInput")
    wcol_d = nc.dram_tensor("wcol", [F, CT_TILES], bf16, kind="ExternalInput")
    ones128_d = nc.dram_tensor("onesff", [F, F], bf16, kind="ExternalInput")
    onest_d = nc.dram_tensor("onest", [2, NC], bf16, kind="ExternalInput")
    out_d = nc.dram_tensor("out", [1, NC], f32, kind="ExternalOutput")
    qcs_d = nc.dram_tensor("qcscratch", [1, C], f32)

    def asf32(ap):
        return ap.bitcast(f32)

    with tile.TileContext(nc) as tc:
        with (
            tc.tile_pool(name="sb", bufs=1) as sb,
            tc.tile_pool(name="phis", bufs=2 * CT_TILES) as phis,
            tc.tile_pool(name="mm", bufs=4, space=bass.MemorySpace.PSUM) as mm,
            tc.tile_pool(name="row", bufs=1, space=bass.MemorySpace.PSUM) as rowp,
            tc.tile_pool(name="ops", bufs=2, space=bass.MemorySpace.PSUM) as ops,
        ):
            # ---- loads (f32r in DRAM and SBUF; bit-identical to f32) ----
            u_sb = sb.tile([F, F], bf16)
            nc.sync.dma_start(u_sb[:], u_d[:])
            ct_sb = sb.tile([F, C], bf16)
            nc.sync.dma_start(ct_sb[:], ct_d[:])
            wcol_sb = sb.tile([F, CT_TILES], bf16)
            nc.gpsimd.dma_start(wcol_sb[:], wcol_d[:])
            xt_sb = sb.tile([F, NC], bf16)
            nc.sync.dma_start(xt_sb[:], xt_d[:])

            onesff = sb.tile([F, F], bf16)
            nc.gpsimd.dma_start(onesff[:], ones128_d[:])
            ones_sb = onesff

            # ---- P = U.T @ U  (= L @ L.T, symmetric) ----
            p_ps = mm.tile([F, F], f32, tag="mm")
            nc.tensor.matmul(p_ps[:], u_sb[:], u_sb[:], start=True, stop=True)
            p_sb = sb.tile([F, F], bf16)
            nc.vector.tensor_copy(p_sb[:], p_ps[:])

            # ---- V = P @ CT; q_c = ones.T @ (V*CT)  -> [1, C] ----
            v_ps = mm.tile([F, C], f32, tag="mm")
            nc.tensor.matmul(v_ps[:], p_sb[:], ct_sb[:], start=True, stop=True)
            vc_sb = sb.tile([F, C], bf16)
            nc.vector.tensor_mul(vc_sb[:], v_ps[:], ct_sb[:])
            qc_ps = rowp.tile([1, C], f32, tag="row")
            nc.tensor.matmul(qc_ps[:], ones_sb[:, 0:1], vc_sb[:], start=True, stop=True)

            # qct[p, t] = -0.5*q_c[t*128+p]  (per-partition exp bias),
            # via DRAM bounce to cross partitions
            qcn_sb = sb.tile([1, C], f32)
            nc.scalar.mul(qcn_sb[:], qc_ps[:], -0.5)
            nc.sync.dma_start(qcs_d[:], qcn_sb[:])
            qct = sb.tile([F, CT_TILES], f32)
            nc.sync.dma_start(
                qct[:, :],
                qcs_d[0:1, :].rearrange("a (t p) -> (a p) t", t=CT_TILES, p=F),
            )

            # ---- per half: W = P @ XT, t = W*XT (from PSUM), qxb bcast,
            # then immediately that half's A/exp/reduce pipeline ----
            w_sb = sb.tile([F, NC], bf16)
            for h in range(NH):
                w_ps = mm.tile([F, 512], f32, tag="mm")
                nc.tensor.matmul(
                    w_ps[:],
                    p_sb[:],
                    xt_sb[:, h * 512 : (h + 1) * 512],
                    start=True,
                    stop=True,
                )
                nc.vector.tensor_copy(w_sb[:, h * 512 : (h + 1) * 512], w_ps[:])
                t_h = sb.tile([F, 512], bf16, tag=f"t{h}")
                nc.vector.tensor_mul(
                    t_h[:], w_ps[:], xt_sb[:, h * 512 : (h + 1) * 512]
                )
                # qxb[p, n] = -0.5*q_x[n] on every partition: all-ones lhsT
                # makes each output row the full partition reduction of t
                qxb_ps = mm.tile([F, 512], f32, tag="mm")
                nc.tensor.matmul(qxb_ps[:], onesff[:], t_h[:], start=True, stop=True)
                qxb = sb.tile([F, 512], bf16, tag=f"qxb{h}")
                nc.vector.tensor_scalar_mul(qxb[:], qxb_ps[:], -0.5)
                qxb_tiles = {h: qxb}
                phi_tiles = []
                for ct in range(CT_TILES):
                    a_ps = mm.tile([128, 512], f32, tag="mm")
                    nc.tensor.matmul(
                        a_ps[:],
                        ct_sb[:, ct * 128 : (ct + 1) * 128],
                        w_sb[:, h * 512 : (h + 1) * 512],
                        start=True,
                        stop=True,
                    )
                    a2 = phis.tile([128, 512], bf16, tag="a2")
                    nc.vector.tensor_add(a2[:], a_ps[:], qxb[:])
                    phi = phis.tile([128, 512], bf16, tag="phi")
                    nc.scalar.activation(
                        phi[:], a2[:], Exp, bias=qct[:, ct : ct + 1]
                    )
                    phi_tiles.append(phi)
                out_ps = ops.tile([1, 512], f32, tag="ops")
                for ct in range(CT_TILES):
                    nc.tensor.matmul(
                        out_ps[:],
                        wcol_sb[:, ct : ct + 1],
                        phi_tiles[ct][:],
                        start=(ct == 0),
                        stop=(ct == CT_TILES - 1),
                    )
                out_sb = sb.tile([1, 512], f32, tag=f"out{h}")
                nc.vector.tensor_copy(out_sb[:], out_ps[:])
                nc.sync.dma_start(out_d[0:1, h * 512 : (h + 1) * 512], out_sb[:])

    nc.compile()
    return nc


def _prep_inputs(X, precision_elements, centers, weights):
    import ml_dtypes

    bf = ml_dtypes.bfloat16
    ti, tj = np.tril_indices(F)
    U = np.zeros((F, F), np.float32)
    U[tj, ti] = precision_elements  # U = L.T
    CT = np.ascontiguousarray(centers.T)
    wcol = np.ascontiguousarray(weights.reshape(CT_TILES, 128).T)
    XT = np.ascontiguousarray(X.T)
    in_maps = []
    for s in range(NCORES):
        in_maps.append(
            {
                "xt": np.ascontiguousarray(XT[:, s * NC : (s + 1) * NC]).astype(bf),
                "u": U.astype(bf),
                "ct": CT.astype(bf),
                "wcol": wcol.astype(bf),
                "onesff": np.ones((F, F), bf),
                "onest": np.ones((2, NC), bf),
            }
        )
    return in_maps


def kernel(X, precision_elements, centers, weights):
    _install_ntff_hook()
    from concourse.bass_utils import run_bass_kernel_spmd

    if "nc" not in _cache:
        _cache["nc"] = _build()
    nc = _cache["nc"]

    in_maps = _prep_inputs(X, precision_elements, centers, weights)
    res = run_bass_kernel_spmd(nc, in_maps, core_ids=list(range(NCORES)))
    _cache["last_results"] = res
    out = np.concatenate([r["out"][0] for r in res.results])
    return out.astype(np.float32)


# revision 11
# speedup vs baseline: 1.0146x; 1.0146x over previous
"""Gaussian RBF network kernel for 8 Trainium2 NeuronCores.

Computes out[n] = sum_c w[c] * exp(-0.5 * (x_n - c_c)^T P (x_n - c_c)),
P = L @ L.T from packed lower-triangular elements, N=8192, C=512, F=128.

Strategy: data-parallel over N (1024 rows per core), everything in
transposed orientation so the final weighted reduction over centers is a
matmul over the partition axis. q_x / q_c fold into the main cross matmul
via a K=2 PSUM-accumulate matmul:
  A[c, n] = sum_k CT[k,c] W[k,n] + 1 * (-0.5 q_x[n]) + (-0.5 q_c[c]) * 1
          = -0.5 (x_n - c_c)^T P (x_n - c_c)
  PhiT = exp(A);  out[n] = sum_c wcol[c] PhiT[c, n]   (w-as-lhsT matmuls)
Matmul operands live as float32r (full-rate fp32 on the PE).
"""

import contextlib
import ctypes
import sys
import types

import numpy as np

N, C, F = 8192, 512, 128
NCORES = 8
NC = N // NCORES  # rows per core
NH = NC // 512  # 512-column n-chunks per core
CT_TILES = C // 128

_cache = {}


def _install_ntff_hook():
    """bass_utils wants antenv.axon_hooks for trace=True under axon; the
    image lacks it. Provide the same ctypes hook trn_boot would install.
    Degrades silently if anything is off (tracing just gets skipped)."""
    if "antenv.axon_hooks" in sys.modules:
        return
    try:
        import antenv

        so_path = "/opt/axon/libaxon_pjrt.so"
        lib = ctypes.CDLL(so_path)
        if not hasattr(lib, "axon_start_nrt_profile"):
            return
        lib.axon_start_nrt_profile.argtypes = [
            ctypes.POINTER(ctypes.c_int64),
            ctypes.c_size_t,
        ]
        lib.axon_start_nrt_profile.restype = ctypes.c_int64
        lib.axon_stop_nrt_profile.argtypes = [ctypes.c_char_p]
        lib.axon_stop_nrt_profile.restype = ctypes.c_int64

        @contextlib.contextmanager
        def _hook(output_dir, device_ids):
            import jax

            jax.devices()
            if device_ids:
                ids = (ctypes.c_int64 * len(device_ids))(*device_ids)
                rc = lib.axon_start_nrt_profile(ids, len(device_ids))
            else:
                rc = lib.axon_start_nrt_profile(None, 0)
            if rc != 0:
                raise RuntimeError(f"axon_start_nrt_profile rc={rc}")
            try:
                yield
            finally:
                n = lib.axon_stop_nrt_profile(str(output_dir).encode())
                if n < 0:
                    raise RuntimeError(f"axon_stop_nrt_profile rc={n}")

        mod = types.ModuleType("antenv.axon_hooks")
        mod.get_axon_ntff_profile_hook = lambda: _hook
        mod.set_axon_ntff_profile_hook = lambda h: None
        sys.modules["antenv.axon_hooks"] = mod
        antenv.axon_hooks = mod
    except Exception:
        pass


def _build():
    import concourse.bass as bass
    import concourse.mybir as mybir
    import concourse.tile as tile
    from concourse import bacc

    f32 = mybir.dt.float32
    bf16 = mybir.dt.bfloat16
    Exp = mybir.ActivationFunctionType.Exp

    nc = bacc.Bacc(
        "TRN2", target_bir_lowering=False, debug=False, num_devices=NCORES
    )
    xt_d = nc.dram_tensor("xt", [F, NC], bf16, kind="ExternalInput")
    u_d = nc.dram_tensor("u", [F, F], bf16, kind="ExternalInput")
    ct_d = nc.dram_tensor("ct", [F, C], bf16, kind="ExternalInput")
    wcol_d = nc.dram_tensor("wcol", [F, CT_TILES], bf16, kind="ExternalInput")
    ones128_d = nc.dram_tensor("onesff", [F, F], bf16, kind="ExternalInput")
    onest_d = nc.dram_tensor("onest", [2, NC], bf16, kind="ExternalInput")
    out_d = nc.dram_tensor("out", [1, NC], f32, kind="ExternalOutput")
    qcs_d = nc.dram_tensor("qcscratch", [1, C], f32)

    def asf32(ap):
        return ap.bitcast(f32)

    with tile.TileContext(nc) as tc:
        with (
            tc.tile_pool(name="sb", bufs=1) as sb,
            tc.tile_pool(name="phis", bufs=2 * CT_TILES) as phis,
            tc.tile_pool(name="mm", bufs=4, space=bass.MemorySpace.PSUM) as mm,
            tc.tile_pool(name="row", bufs=1, space=bass.MemorySpace.PSUM) as rowp,
            tc.tile_pool(name="ops", bufs=2, space=bass.MemorySpace.PSUM) as ops,
        ):
            # ---- loads (f32r in DRAM and SBUF; bit-identical to f32) ----
            u_sb = sb.tile([F, F], bf16)
            nc.sync.dma_start(u_sb[:], u_d[:])
            ct_sb = sb.tile([F, C], bf16)
            nc.sync.dma_start(ct_sb[:], ct_d[:])
            wcol_sb = sb.tile([F, CT_TILES], bf16)
            nc.gpsimd.dma_start(wcol_sb[:], wcol_d[:])
            xt_sb = sb.tile([F, NC], bf16)
            nc.sync.dma_start(xt_sb[:], xt_d[:])

            onesff = sb.tile([F, F], bf16)
            nc.gpsimd.dma_start(onesff[:], ones128_d[:])
            ones_sb = onesff

            # ---- P = U.T @ U  (= L @ L.T, symmetric) ----
            p_ps = mm.tile([F, F], f32, tag="mm")
            nc.tensor.matmul(p_ps[:], u_sb[:], u_sb[:], start=True, stop=True)
            p_sb = sb.tile([F, F], bf16)
            nc.vector.tensor_copy(p_sb[:], p_ps[:])

            # ---- V = P @ CT; q_c = ones.T @ (V*CT)  -> [1, C] ----
            v_ps = mm.tile([F, C], f32, tag="mm")
            nc.tensor.matmul(v_ps[:], p_sb[:], ct_sb[:], start=True, stop=True)
            vc_sb = sb.tile([F, C], bf16)
            nc.vector.tensor_mul(vc_sb[:], v_ps[:], ct_sb[:])
            qc_ps = rowp.tile([1, C], f32, tag="row")
            nc.tensor.matmul(qc_ps[:], ones_sb[:, 0:1], vc_sb[:], start=True, stop=True)

            # qct[p, t] = -0.5*q_c[t*128+p]  (per-partition exp bias),
            # via DRAM bounce to cross partitions
            qcn_sb = sb.tile([1, C], f32)
            nc.scalar.mul(qcn_sb[:], qc_ps[:], -0.5)
            nc.sync.dma_start(qcs_d[:], qcn_sb[:])
            qct = sb.tile([F, CT_TILES], f32)
            nc.sync.dma_start(
                qct[:, :],
                qcs_d[0:1, :].rearrange("a (t p) -> (a p) t", t=CT_TILES, p=F),
            )

            # ---- W = P @ XT  -> [F, NC] ----
            w_sb = sb.tile([F, NC], bf16)
            for h in range(NH):
                w_ps = mm.tile([F, 512], f32, tag="mm")
                nc.tensor.matmul(
                    w_ps[:],
                    p_sb[:],
                    xt_sb[:, h * 512 : (h + 1) * 512],
                    start=True,
                    stop=True,
                )
                nc.vector.tensor_copy(w_sb[:, h * 512 : (h + 1) * 512], w_ps[:])

            # ---- q_x = ones.T @ (W*XT) -> [1, NC] ----
            t_sb = sb.tile([F, NC], bf16)
            nc.vector.tensor_mul(t_sb[:], w_sb[:], xt_sb[:])
            # qxb[p, n] = -0.5*q_x[n] on every partition: all-ones lhsT makes
            # each output row the full partition reduction of t; scale by -0.5
            # during the PSUM->SBUF copy
            qxb_tiles = []
            for h in range(NH):
                qxb_ps = mm.tile([F, 512], f32, tag="mm")
                nc.tensor.matmul(
                    qxb_ps[:],
                    onesff[:],
                    t_sb[:, h * 512 : (h + 1) * 512],
                    start=True,
                    stop=True,
                )
                qxb = sb.tile([F, 512], bf16, tag=f"qxb{h}")
                nc.vector.tensor_scalar_mul(qxb[:], qxb_ps[:], -0.5)
                qxb_tiles.append(qxb)

            # ---- per n-chunk: A tiles, exp, weighted reduce over c ----
            for h in range(NH):
                phi_tiles = []
                for ct in range(CT_TILES):
                    a_ps = mm.tile([128, 512], f32, tag="mm")
                    nc.tensor.matmul(
                        a_ps[:],
                        ct_sb[:, ct * 128 : (ct + 1) * 128],
                        w_sb[:, h * 512 : (h + 1) * 512],
                        start=True,
                        stop=True,
                    )
                    a2 = phis.tile([128, 512], bf16, tag="a2")
                    nc.vector.tensor_add(a2[:], a_ps[:], qxb_tiles[h][:])
                    phi = phis.tile([128, 512], bf16, tag="phi")
                    nc.scalar.activation(
                        phi[:], a2[:], Exp, bias=qct[:, ct : ct + 1]
                    )
                    phi_tiles.append(phi)
                out_ps = ops.tile([1, 512], f32, tag="ops")
                for ct in range(CT_TILES):
                    nc.tensor.matmul(
                        out_ps[:],
                        wcol_sb[:, ct : ct + 1],
                        phi_tiles[ct][:],
                        start=(ct == 0),
                        stop=(ct == CT_TILES - 1),
                    )
                out_sb = sb.tile([1, 512], f32, tag=f"out{h}")
                nc.vector.tensor_copy(out_sb[:], out_ps[:])
                nc.sync.dma_start(out_d[0:1, h * 512 : (h + 1) * 512], out_sb[:])

    nc.compile()
    return nc


def _prep_inputs(X, precision_elements, centers, weights):
    import ml_dtypes

    bf = ml_dtypes.bfloat16
    ti, tj = np.tril_indices(F)
    U = np.zeros((F, F), np.float32)
    U[tj, ti] = precision_elements  # U = L.T
    CT = np.ascontiguousarray(centers.T)
    wcol = np.ascontiguousarray(weights.reshape(CT_TILES, 128).T)
    XT = np.ascontiguousarray(X.T)
    in_maps = []
    for s in range(NCORES):
        in_maps.append(
            {
                "xt": np.ascontiguousarray(XT[:, s * NC : (s + 1) * NC]).astype(bf),
                "u": U.astype(bf),
                "ct": CT.astype(bf),
                "wcol": wcol.astype(bf),
                "onesff": np.ones((F, F), bf),
                "onest": np.ones((2, NC), bf),
            }
        )
    return in_maps


def kernel(X, precision_elements, centers, weights):
    _install_ntff_hook()
    from concourse.bass_utils import run_bass_kernel_spmd

    if "nc" not in _cache:
        _cache["nc"] = _build()
    nc = _cache["nc"]

    in_maps = _prep_inputs(X, precision_elements, centers, weights)
    res = run_bass_kernel_spmd(nc, in_maps, core_ids=list(range(NCORES)))
    _cache["last_results"] = res
    out = np.concatenate([r["out"][0] for r in res.results])
    return out.astype(np.float32)
